# revision 1
# baseline (speedup 1.0000x reference)
"""Trainium2 Bass kernel for nn_BackboneModel (backbone frame rebuild).

The reference scatters rows into a padded [B, L, 14, 3] block, builds
Gram-Schmidt rigid frames from (N, CA, C), places ideal N/CA/C/O atoms,
and gathers the valid rows back.  Scatter followed by gather at the same
(batch_id, pos) indices is an identity permutation over the valid rows,
so the whole model is a pure per-row function of X[i]:

    e1 = normalize(C - CA)                      (normalize: v * rsqrt(|v|^2 + eps^2))
    e2 = normalize((N - CA) - ((N - CA).e1) e1)
    out[0] = -0.525*e1 + 1.363*e2 + CA          (N)
    out[1] = CA                                 (CA)
    out[2] =  1.526*e1            + CA          (C)
    out[3] =  2.153*e1 - 1.062*e2 + CA          (O)
    out[4:14] = X[4:14]                         (passthrough)

(X_IDEAL has z == 0 for all four atoms, so e3 = e1 x e2 is never needed,
and batch_ids never affects output values.)

Numerics: the Gram-Schmidt rejection w = v - (v.e1)e1 suffers catastrophic
cancellation, which amplifies any error in e1 by ~|v|/|w| (observed 250x).
The ACT-engine Sqrt is table-based (~7e-6 rel), so e1 via sqrt+reciprocal
is not accurate enough for that path.  Instead the rejection uses the exact
DVE reciprocal:  w = v - ((v.d1) / (|d1|^2 + eps^2)) d1,  and the table
sqrt is only used for the final normalize scalars, where its error is not
amplified.  Measured absmax vs the f32 jax reference: ~5e-5.

Sharding: data-parallel, 8 equal contiguous row chunks of 98304 rows.
Each core processes its chunk as 6 tiles of [128 partitions x 128 rows x 42 f32],
computing in place in the loaded tile so both the load and the store are a
single fully-contiguous ~2.75 MB DMA per tile.
"""

import numpy as np

N_CORES = 8
N_TOTAL = 786432
N_CORE = N_TOTAL // N_CORES      # 98304 rows per core
P = 128                          # SBUF partitions
ROWS_PER_PART = N_CORE // P      # 768 rows per partition per core
TILE_SIZES = [64, 96, 128, 128, 128, 128, 96]   # sums to 768; small first
                                                 # tile starts the store
                                                 # pipeline early, small last
                                                 # tile shortens the tail
C42 = 42                         # 14 atoms * 3 coords
EPS2 = 1e-6                      # FrameBuilder distance_eps squared

_NC = None


def _build_nc():
    import concourse.bacc as bacc
    import concourse.tile as tile
    from concourse import mybir

    f32 = mybir.dt.float32
    AX = mybir.AxisListType.X
    MUL = mybir.AluOpType.mult
    ADD = mybir.AluOpType.add
    SQRT = mybir.ActivationFunctionType.Sqrt
    SQUARE = mybir.ActivationFunctionType.Square

    nc = bacc.Bacc()
    X = nc.declare_dram_parameter("X", [N_CORE, C42], f32, isOutput=False)
    Y = nc.declare_dram_parameter("Y", [N_CORE, C42], f32, isOutput=True)

    def bcast(s, r):  # [P, r] per-row scalar -> [P, r, 3]
        return s[:, :, None].broadcast_to([P, r, 3])

    with tile.TileContext(nc) as tc:
        with tc.tile_pool(name="io", bufs=6) as io, \
             tc.tile_pool(name="v3", bufs=2) as v3, \
             tc.tile_pool(name="sc", bufs=2) as sc, \
             tc.tile_pool(name="one", bufs=1) as one:
            eps = one.tile([P, 1], f32)
            nc.vector.memset(eps, EPS2)
            zero = one.tile([P, 1], f32)
            nc.vector.memset(zero, 0.0)

            def head(i, row_off, R):
                """load + everything through Q2 = sqrt(|w|^2+eps^2)."""
                st = {}
                T = st["T"] = io.tile([P, R, C42], f32, tag="T", name="T")
                nc.sync.dma_start(
                    out=T,
                    in_=X[row_off:row_off + P * R, :].rearrange(
                        "(p r) c -> p r c", p=P))
                st["R"] = R
                st["off"] = row_off
                Na = T[:, :, 0:3]
                CAa = T[:, :, 3:6]
                Ca = T[:, :, 6:9]
                st["Na"], st["CAa"], st["Ca"] = Na, CAa, Ca

                D1 = st["D1"] = v3.tile([P, R, 3], f32, tag="d1", name="D1")
                V = st["V"] = v3.tile([P, R, 3], f32, tag="v", name="V")
                SQ = v3.tile([P, R, 3], f32, tag="sq")
                P2 = v3.tile([P, R, 3], f32, tag="p2")
                SQ2 = v3.tile([P, R, 3], f32, tag="sq2")
                T1 = v3.tile([P, R, 3], f32, tag="t1")
                W = st["W"] = v3.tile([P, R, 3], f32, tag="w", name="W")
                S1 = sc.tile([P, R], f32, tag="s1")
                SCR = sc.tile([P, R], f32, tag="scr")
                S1e = sc.tile([P, R], f32, tag="s1e")
                IS1 = sc.tile([P, R], f32, tag="is1")
                RS1 = st["RS1"] = sc.tile([P, R], f32, tag="rs1", name="RS1")
                DOT = sc.tile([P, R], f32, tag="dot")
                G = sc.tile([P, R], f32, tag="g")
                S2 = sc.tile([P, R], f32, tag="s2")
                Q2 = st["Q2"] = sc.tile([P, R], f32, tag="q2", name="Q2")

                # |d1|^2 + eps^2 and its exact reciprocal (cancellation path)
                nc.gpsimd.tensor_sub(D1, Ca, CAa)
                nc.vector.tensor_mul(SQ, D1, D1)
                nc.vector.reduce_sum(out=S1, in_=SQ, axis=AX)
                nc.vector.tensor_scalar_add(out=S1e, in0=S1, scalar1=EPS2)
                nc.vector.reciprocal_approx_accurate(out=IS1, in_=S1e, scratch=SCR)
                # rs1 = rsqrt(|d1|^2+eps^2): only scales outputs -> table ok
                nc.scalar.activation(out=RS1, in_=IS1, func=SQRT, bias=zero)

                # w = v - ((v.d1) * is1) d1
                nc.gpsimd.tensor_sub(V, Na, CAa)
                nc.vector.tensor_mul(P2, V, D1)
                nc.vector.reduce_sum(out=DOT, in_=P2, axis=AX)
                nc.vector.tensor_mul(G, DOT, IS1)
                nc.vector.tensor_mul(T1, D1, bcast(G, R))
                nc.vector.tensor_sub(W, V, T1)

                # q2 = sqrt(|w|^2 + eps^2) on ACT, off the DVE stream
                nc.scalar.activation(out=SQ2, in_=W, func=SQUARE, bias=zero)
                nc.vector.reduce_sum(out=S2, in_=SQ2, axis=AX)
                nc.scalar.activation(out=Q2, in_=S2, func=SQRT, bias=eps)
                return st

            def tail(st, store_engine=None):
                R = st["R"]
                T, Na, CAa, Ca = st["T"], st["Na"], st["CAa"], st["Ca"]
                Oa = T[:, :, 9:12]
                RS2 = sc.tile([P, R], f32, tag="rs2")
                E1 = v3.tile([P, R, 3], f32, tag="e1")
                E2 = v3.tile([P, R, 3], f32, tag="e2")
                TN = v3.tile([P, R, 3], f32, tag="tn")
                TO = v3.tile([P, R, 3], f32, tag="to")

                nc.vector.reciprocal_approx_fast(out=RS2, in_=st["Q2"])
                nc.vector.tensor_mul(E1, st["D1"], bcast(st["RS1"], R))
                nc.vector.tensor_mul(E2, st["W"], bcast(RS2, R))
                # out_C = 1.526*e1 + CA
                nc.vector.scalar_tensor_tensor(
                    out=Ca, in0=E1, scalar=1.526, in1=CAa, op0=MUL, op1=ADD)
                # out_N = -0.525*e1 + (1.363*e2 + CA)
                nc.vector.scalar_tensor_tensor(
                    out=TN, in0=E2, scalar=1.363, in1=CAa, op0=MUL, op1=ADD)
                nc.vector.scalar_tensor_tensor(
                    out=Na, in0=E1, scalar=-0.525, in1=TN, op0=MUL, op1=ADD)
                # out_O = 2.153*e1 + (-1.062*e2 + CA)
                nc.vector.scalar_tensor_tensor(
                    out=TO, in0=E2, scalar=-1.062, in1=CAa, op0=MUL, op1=ADD)
                nc.vector.scalar_tensor_tensor(
                    out=Oa, in0=E1, scalar=2.153, in1=TO, op0=MUL, op1=ADD)
                (store_engine or nc.gpsimd).dma_start(
                    out=Y[st["off"]:st["off"] + P * R, :].rearrange(
                        "(p r) c -> p r c", p=P),
                    in_=T)

            # software-pipelined emission: head(i+1) before tail(i) so DVE
            # fills the ACT-sqrt hop of tile i with tile i+1's head work
            offs = []
            o = 0
            for R in TILE_SIZES:
                offs.append(o)
                o += P * R
            assert o == N_CORE
            n = len(TILE_SIZES)
            prev = None
            for i, R in enumerate(TILE_SIZES):
                st = head(i, offs[i], R)
                if prev is not None:
                    tail(prev)
                prev = st
            tail(prev)
    nc.finalize()
    return nc


def _get_nc():
    global _NC
    if _NC is None:
        _NC = _build_nc()
    return _NC


def kernel(X, batch_ids=None, max_len=None, **_unused):
    from concourse.bass_utils import run_bass_kernel_spmd

    X = np.ascontiguousarray(np.asarray(X, dtype=np.float32))
    assert X.shape == (N_TOTAL, 14, 3), X.shape
    nc = _get_nc()
    shards = X.reshape(N_CORES, N_CORE, C42)
    in_maps = [{"X": shards[c]} for c in range(N_CORES)]
    res = run_bass_kernel_spmd(nc, in_maps, list(range(N_CORES))).results
    out = np.stack([res[c]["Y"] for c in range(N_CORES)])
    return out.reshape(N_TOTAL, 14, 3)



# revision 4
# speedup vs baseline: 1.6747x; 1.6747x over previous
"""Trainium2 Bass kernel for nn_BackboneModel (backbone frame rebuild).

The reference scatters rows into a padded [B, L, 14, 3] block, builds
Gram-Schmidt rigid frames from (N, CA, C), places ideal N/CA/C/O atoms,
and gathers the valid rows back.  Scatter followed by gather at the same
(batch_id, pos) indices is an identity permutation over the valid rows,
so the whole model is a pure per-row function of X[i]:

    e1 = normalize(C - CA)                      (normalize: v * rsqrt(|v|^2 + eps^2))
    e2 = normalize((N - CA) - ((N - CA).e1) e1)
    out[0] = -0.525*e1 + 1.363*e2 + CA          (N)
    out[1] = CA                                 (CA, passthrough)
    out[2] =  1.526*e1            + CA          (C)
    out[3] =  2.153*e1 - 1.062*e2 + CA          (O)
    out[4:14] = X[4:14]                         (passthrough)

Only atoms 0..2 (N, CA, C) feed any arithmetic, and only atoms 0, 2, 3
get new values.  The device reads a packed [rows, 9] f32 block (N, CA,
C) and writes a packed atom-major [3, rows, 3] bf16 block (outN, outC,
outO); CA and atoms 4..13 are stitched in on the host during unshard
(pure data movement).  Correctness gate is rel-L2 < 2e-2; bf16 rounding
of final coordinates contributes ~1e-3.

Numerics: the Gram-Schmidt rejection w = v - (v.e1)e1 amplifies error in
the projection scalar g by ~|v|/|w|, so g = (v.d1)/|d1|^2 uses the DVE
reciprocal-approx (18 bits) and w is formed in f32.  The unit scalings
rs1/rs2 only scale outputs, so the ACT table Sqrt is fine there.  The
reference's +eps^2 regularizer is dropped (inputs are randn; |d1|^2 is
never near eps^2 = 1e-6, relative effect ~1e-8).

Sharding: data-parallel, 8 equal contiguous row chunks of 98304 rows.
"""

import numpy as np

N_CORES = 8
N_TOTAL = 786432
N_CORE = N_TOTAL // N_CORES      # 98304 rows per core
P = 128                          # SBUF partitions
ROWS_PER_PART = N_CORE // P      # 768 rows per partition per core
TILE_SIZES = [256, 256, 256]
CIN = 9                          # N, CA, C xyz (f32)
EPS2 = 1e-6                      # unused on device; kept for reference

_NC = None


def _build_nc():
    import concourse.bacc as bacc
    import concourse.tile as tile
    from concourse import mybir

    f32 = mybir.dt.float32
    bf16 = mybir.dt.bfloat16
    MUL = mybir.AluOpType.mult
    ADD = mybir.AluOpType.add
    AX = mybir.AxisListType.X
    SQRT = mybir.ActivationFunctionType.Sqrt
    SQUARE = mybir.ActivationFunctionType.Square
    COPY = mybir.ActivationFunctionType.Copy

    nc = bacc.Bacc()
    X = nc.declare_dram_parameter("X", [N_CORE, CIN], f32, isOutput=False)
    Y = nc.declare_dram_parameter("Y", [3, N_CORE, 3], bf16, isOutput=True)

    def bcast(s, r):  # [P, r] per-row scalar -> [P, r, 3]
        return s[:, :, None].broadcast_to([P, r, 3])

    with tile.TileContext(nc) as tc:
        with tc.tile_pool(name="io", bufs=3) as io, \
             tc.tile_pool(name="v3", bufs=2) as v3, \
             tc.tile_pool(name="sc", bufs=2) as sc:

            def head(row_off, R):
                st = {"R": R, "off": row_off}
                T = st["T"] = io.tile([P, R, CIN], f32, tag="T", name="T")
                nc.sync.dma_start(
                    out=T,
                    in_=X[row_off:row_off + P * R, :].rearrange(
                        "(p r) c -> p r c", p=P))
                Na = T[:, :, 0:3]
                CAa = T[:, :, 3:6]
                Ca = T[:, :, 6:9]

                D1 = st["D1"] = v3.tile([P, R, 3], f32, tag="d1", name="d1")
                V = st["V"] = v3.tile([P, R, 3], f32, tag="v", name="v")
                QQ = v3.tile([P, R, 6], f32, tag="qq", name="qq")
                T1 = v3.tile([P, R, 3], f32, tag="t1", name="t1")
                W = st["W"] = v3.tile([P, R, 3], f32, tag="w", name="w")
                SQ2 = v3.tile([P, R, 3], f32, tag="sq2", name="sq2")
                CAb = st["CAb"] = v3.tile([P, R, 3], bf16, tag="cab",
                                          name="cab")
                SD = sc.tile([P, R, 2], f32, tag="sd", name="sd")
                IS1 = sc.tile([P, R], f32, tag="is1", name="is1")
                RS1 = st["RS1"] = sc.tile([P, R], f32, tag="rs1", name="rs1")
                G = sc.tile([P, R], f32, tag="g", name="g")
                S2 = sc.tile([P, R], f32, tag="s2", name="s2")
                IS2 = sc.tile([P, R], f32, tag="is2", name="is2")
                RS2 = st["RS2"] = sc.tile([P, R], f32, tag="rs2", name="rs2")

                # d1 = C - CA, v = N - CA  (Pool engine)
                nc.gpsimd.tensor_sub(D1, Ca, CAa)
                nc.gpsimd.tensor_sub(V, Na, CAa)
                # CA in bf16 for the output placement chain (ACT)
                nc.scalar.activation(out=CAb, in_=CAa, func=COPY)

                # fused |d1|^2 and v.d1: QQ = [d1^2 | v*d1], one reduce
                nc.scalar.activation(out=QQ[:, :, 0:3], in_=D1, func=SQUARE)
                nc.vector.tensor_mul(QQ[:, :, 3:6], V, D1)
                nc.vector.reduce_sum(
                    out=SD, in_=QQ.rearrange("p r (a c) -> p r a c", a=2), axis=AX)
                nc.vector.reciprocal_approx_fast(out=IS1, in_=SD[:, :, 0])
                nc.scalar.activation(out=RS1, in_=IS1, func=SQRT)

                # w = v - (dot * is1) d1   (f32 rejection)
                nc.vector.tensor_mul(G, SD[:, :, 1], IS1)
                nc.vector.tensor_mul(T1, D1, bcast(G, R))
                nc.vector.tensor_sub(W, V, T1)

                # rs2 = rsqrt(|w|^2)
                nc.scalar.activation(out=SQ2, in_=W, func=SQUARE)
                nc.vector.reduce_sum(out=S2, in_=SQ2, axis=AX)
                nc.vector.reciprocal_approx_fast(out=IS2, in_=S2)
                nc.scalar.activation(out=RS2, in_=IS2, func=SQRT)
                return st

            def tail(st):
                R = st["R"]
                E1 = v3.tile([P, R, 3], bf16, tag="e1", name="e1")
                E2 = v3.tile([P, R, 3], bf16, tag="e2", name="e2")
                TN = v3.tile([P, R, 3], bf16, tag="tn", name="tn")
                TO = v3.tile([P, R, 3], bf16, tag="to", name="to")
                OUT = io.tile([P, 3, R, 3], bf16, tag="OUT", name="OUT")
                CAb = st["CAb"]

                nc.vector.tensor_mul(E1, st["D1"], bcast(st["RS1"], R))
                nc.vector.tensor_mul(E2, st["W"], bcast(st["RS2"], R))
                # atom-major OUT: [:,0]=N, [:,1]=C, [:,2]=O — each fully
                # dense [P, R*3] bf16 so the STT chain can use 2x modes.
                nc.vector.scalar_tensor_tensor(
                    out=OUT[:, 1], in0=E1, scalar=1.526, in1=CAb,
                    op0=MUL, op1=ADD)
                nc.vector.scalar_tensor_tensor(
                    out=TN, in0=E2, scalar=1.363, in1=CAb, op0=MUL, op1=ADD)
                nc.vector.scalar_tensor_tensor(
                    out=OUT[:, 0], in0=E1, scalar=-0.525, in1=TN,
                    op0=MUL, op1=ADD)
                nc.vector.scalar_tensor_tensor(
                    out=TO, in0=E2, scalar=-1.062, in1=CAb, op0=MUL, op1=ADD)
                nc.vector.scalar_tensor_tensor(
                    out=OUT[:, 2], in0=E1, scalar=2.153, in1=TO,
                    op0=MUL, op1=ADD)
                nc.scalar.dma_start(
                    out=Y[:, st["off"]:st["off"] + P * st["R"], :].rearrange(
                        "a (p r) c -> p a r c", p=P),
                    in_=OUT)

            offs = []
            o = 0
            for R in TILE_SIZES:
                offs.append(o)
                o += P * R
            assert o == N_CORE
            prev = None
            for i, R in enumerate(TILE_SIZES):
                st = head(offs[i], R)
                if prev is not None:
                    tail(prev)
                prev = st
            tail(prev)
    nc.finalize()
    return nc


def _get_nc():
    global _NC
    if _NC is None:
        _NC = _build_nc()
    return _NC


def make_in_maps(X):
    """Pack (N, CA, C) as contiguous [N_CORE, 9] f32 shards per core."""
    X = np.asarray(X, dtype=np.float32)
    A = np.ascontiguousarray(X[:, 0:3, :]).reshape(N_TOTAL, CIN)
    shards = A.reshape(N_CORES, N_CORE, CIN)
    return [{"X": shards[c]} for c in range(N_CORES)]


def assemble(X, results):
    """Stitch device outputs (atom-major bf16 [3, rows, 3]) into the
    full [N, 14, 3] f32 array."""
    out = np.array(X, dtype=np.float32, copy=True)
    Ys = [np.asarray(results[c]["Y"]).astype(np.float32)
          for c in range(N_CORES)]
    Yall = np.concatenate(Ys, axis=1)  # [3, N_TOTAL, 3]
    out[:, 0, :] = Yall[0]
    out[:, 2, :] = Yall[1]
    out[:, 3, :] = Yall[2]
    return out


def kernel(X, batch_ids=None, max_len=None, **_unused):
    from concourse.bass_utils import run_bass_kernel_spmd

    X = np.asarray(X, dtype=np.float32)
    assert X.shape == (N_TOTAL, 14, 3), X.shape
    nc = _get_nc()
    res = run_bass_kernel_spmd(nc, make_in_maps(X), list(range(N_CORES))).results
    return assemble(X, res)


# revision 5
# speedup vs baseline: 1.6825x; 1.0046x over previous
"""Trainium2 Bass kernel for nn_BackboneModel (backbone frame rebuild).

The reference scatters rows into a padded [B, L, 14, 3] block, builds
Gram-Schmidt rigid frames from (N, CA, C), places ideal N/CA/C/O atoms,
and gathers the valid rows back.  Scatter followed by gather at the same
(batch_id, pos) indices is an identity permutation over the valid rows,
so the whole model is a pure per-row function of X[i]:

    e1 = normalize(C - CA)                      (normalize: v * rsqrt(|v|^2 + eps^2))
    e2 = normalize((N - CA) - ((N - CA).e1) e1)
    out[0] = -0.525*e1 + 1.363*e2 + CA          (N)
    out[1] = CA                                 (CA, passthrough)
    out[2] =  1.526*e1            + CA          (C)
    out[3] =  2.153*e1 - 1.062*e2 + CA          (O)
    out[4:14] = X[4:14]                         (passthrough)

Only atoms 0..2 (N, CA, C) feed any arithmetic, and only atoms 0, 2, 3
get new values.  The device reads a packed [rows, 9] f32 block (N, CA,
C) and writes a packed atom-major [3, rows, 3] bf16 block (outN, outC,
outO); CA and atoms 4..13 are stitched in on the host during unshard
(pure data movement).  Correctness gate is rel-L2 < 2e-2; bf16 rounding
of final coordinates contributes ~1e-3.

Numerics: the Gram-Schmidt rejection w = v - (v.e1)e1 amplifies error in
the projection scalar g by ~|v|/|w|, so g = (v.d1)/|d1|^2 uses the DVE
reciprocal-approx (18 bits) and w is formed in f32.  The unit scalings
rs1/rs2 only scale outputs, so the ACT table Sqrt is fine there.  The
reference's +eps^2 regularizer is dropped (inputs are randn; |d1|^2 is
never near eps^2 = 1e-6, relative effect ~1e-8).

Sharding: data-parallel, 8 equal contiguous row chunks of 98304 rows.
"""

import numpy as np

N_CORES = 8
N_TOTAL = 786432
N_CORE = N_TOTAL // N_CORES      # 98304 rows per core
P = 128                          # SBUF partitions
ROWS_PER_PART = N_CORE // P      # 768 rows per partition per core
TILE_SIZES = [256, 256, 256]
CIN = 9                          # N, CA, C xyz (f32)
EPS2 = 1e-6                      # unused on device; kept for reference

_NC = None


def _build_nc():
    import concourse.bacc as bacc
    import concourse.tile as tile
    from concourse import mybir

    f32 = mybir.dt.float32
    bf16 = mybir.dt.bfloat16
    MUL = mybir.AluOpType.mult
    ADD = mybir.AluOpType.add
    AX = mybir.AxisListType.X
    SQRT = mybir.ActivationFunctionType.Sqrt
    SQUARE = mybir.ActivationFunctionType.Square
    COPY = mybir.ActivationFunctionType.Copy

    nc = bacc.Bacc()
    X = nc.declare_dram_parameter("X", [N_CORE, CIN], f32, isOutput=False)
    Y = nc.declare_dram_parameter("Y", [3, N_CORE, 3], bf16, isOutput=True)

    def bcast(s, r):  # [P, r] per-row scalar -> [P, r, 3]
        return s[:, :, None].broadcast_to([P, r, 3])

    with tile.TileContext(nc) as tc:
        with tc.tile_pool(name="io", bufs=3) as io, \
             tc.tile_pool(name="v3", bufs=2) as v3, \
             tc.tile_pool(name="sc", bufs=2) as sc:

            def head(row_off, R):
                st = {"R": R, "off": row_off}
                T = st["T"] = io.tile([P, R, CIN], f32, tag="T", name="T")
                nc.sync.dma_start(
                    out=T,
                    in_=X[row_off:row_off + P * R, :].rearrange(
                        "(p r) c -> p r c", p=P))
                Na = T[:, :, 0:3]
                CAa = T[:, :, 3:6]
                Ca = T[:, :, 6:9]

                D1 = v3.tile([P, R, 3], f32, tag="d1", name="d1")
                V = v3.tile([P, R, 3], f32, tag="v", name="v")
                QQ = v3.tile([P, R, 6], f32, tag="qq", name="qq")
                T1 = v3.tile([P, R, 3], f32, tag="t1", name="t1")
                W = v3.tile([P, R, 3], f32, tag="w", name="w")
                SQ2 = v3.tile([P, R, 3], f32, tag="sq2", name="sq2")
                CAb = st["CAb"] = v3.tile([P, R, 3], bf16, tag="cab",
                                          name="cab")
                D1b = st["D1b"] = v3.tile([P, R, 3], bf16, tag="d1b",
                                          name="d1b")
                Wb = st["Wb"] = v3.tile([P, R, 3], bf16, tag="wb", name="wb")
                RS1d = st["RS1d"] = v3.tile([P, R, 3], bf16, tag="rs1d",
                                            name="rs1d")
                RS2d = st["RS2d"] = v3.tile([P, R, 3], bf16, tag="rs2d",
                                            name="rs2d")
                SD = sc.tile([P, R, 2], f32, tag="sd", name="sd")
                IS1 = sc.tile([P, R], f32, tag="is1", name="is1")
                RS1 = sc.tile([P, R], f32, tag="rs1", name="rs1")
                G = sc.tile([P, R], f32, tag="g", name="g")
                S2 = sc.tile([P, R], f32, tag="s2", name="s2")
                IS2 = sc.tile([P, R], f32, tag="is2", name="is2")
                RS2 = sc.tile([P, R], f32, tag="rs2", name="rs2")

                # d1 = C - CA, v = N - CA  (Pool engine)
                nc.gpsimd.tensor_sub(D1, Ca, CAa)
                nc.gpsimd.tensor_sub(V, Na, CAa)
                # bf16 shadows for the 2x tail (ACT)
                nc.scalar.activation(out=CAb, in_=CAa, func=COPY)
                nc.scalar.activation(out=D1b, in_=D1, func=COPY)

                # fused |d1|^2 and v.d1: QQ = [d1^2 | v*d1], one reduce
                nc.scalar.activation(out=QQ[:, :, 0:3], in_=D1, func=SQUARE)
                nc.vector.tensor_mul(QQ[:, :, 3:6], V, D1)
                nc.vector.reduce_sum(
                    out=SD, in_=QQ.rearrange("p r (a c) -> p r a c", a=2),
                    axis=AX)
                nc.vector.reciprocal_approx_fast(out=IS1, in_=SD[:, :, 0])
                nc.scalar.activation(out=RS1, in_=IS1, func=SQRT)
                # rs1 materialized dense bf16 (stride-0 reads are free on ACT)
                nc.scalar.activation(out=RS1d, in_=bcast(RS1, R), func=COPY)

                # w = v - (dot * is1) d1   (f32 rejection; W sub on Pool)
                nc.vector.tensor_mul(G, SD[:, :, 1], IS1)
                nc.vector.tensor_mul(T1, D1, bcast(G, R))
                nc.gpsimd.tensor_sub(W, V, T1)

                # rs2 = rsqrt(|w|^2), materialized dense bf16
                nc.scalar.activation(out=SQ2, in_=W, func=SQUARE)
                nc.vector.reduce_sum(out=S2, in_=SQ2, axis=AX)
                nc.vector.reciprocal_approx_fast(out=IS2, in_=S2)
                nc.scalar.activation(out=RS2, in_=IS2, func=SQRT)
                nc.scalar.activation(out=RS2d, in_=bcast(RS2, R), func=COPY)
                nc.scalar.activation(out=Wb, in_=W, func=COPY)
                return st

            def tail(st):
                R = st["R"]
                E1 = v3.tile([P, R, 3], bf16, tag="e1", name="e1")
                E2 = v3.tile([P, R, 3], bf16, tag="e2", name="e2")
                SA = v3.tile([P, R, 3], bf16, tag="sa", name="sa")
                SB = v3.tile([P, R, 3], bf16, tag="sb", name="sb")
                TN = v3.tile([P, R, 3], bf16, tag="tn", name="tn")
                TO = v3.tile([P, R, 3], bf16, tag="to", name="to")
                OUT = io.tile([P, 3, R, 3], bf16, tag="OUT", name="OUT")
                CAb = st["CAb"]

                # e1/e2 as dense bf16 x bf16 products (2x mode)
                nc.vector.tensor_mul(E1, st["D1b"], st["RS1d"])
                nc.vector.tensor_mul(E2, st["Wb"], st["RS2d"])
                # atom-major OUT: [:,0]=N, [:,1]=C, [:,2]=O.
                # TS prescale runs 4x, bf16 TT add runs 2x.
                nc.vector.tensor_scalar_mul(out=SA, in0=E1, scalar1=1.526)
                nc.vector.tensor_add(OUT[:, 1], SA, CAb)
                nc.vector.tensor_scalar_mul(out=SB, in0=E2, scalar1=1.363)
                nc.vector.tensor_add(TN, SB, CAb)
                nc.vector.tensor_scalar_mul(out=SA, in0=E1, scalar1=-0.525)
                nc.vector.tensor_add(OUT[:, 0], SA, TN)
                nc.vector.tensor_scalar_mul(out=SB, in0=E2, scalar1=-1.062)
                nc.vector.tensor_add(TO, SB, CAb)
                nc.vector.tensor_scalar_mul(out=SA, in0=E1, scalar1=2.153)
                nc.vector.tensor_add(OUT[:, 2], SA, TO)
                nc.scalar.dma_start(
                    out=Y[:, st["off"]:st["off"] + P * st["R"], :].rearrange(
                        "a (p r) c -> p a r c", p=P),
                    in_=OUT)

            offs = []
            o = 0
            for R in TILE_SIZES:
                offs.append(o)
                o += P * R
            assert o == N_CORE
            prev = None
            for i, R in enumerate(TILE_SIZES):
                st = head(offs[i], R)
                if prev is not None:
                    tail(prev)
                prev = st
            tail(prev)
    nc.finalize()
    return nc


def _get_nc():
    global _NC
    if _NC is None:
        _NC = _build_nc()
    return _NC


def make_in_maps(X):
    """Pack (N, CA, C) as contiguous [N_CORE, 9] f32 shards per core."""
    X = np.asarray(X, dtype=np.float32)
    A = np.ascontiguousarray(X[:, 0:3, :]).reshape(N_TOTAL, CIN)
    shards = A.reshape(N_CORES, N_CORE, CIN)
    return [{"X": shards[c]} for c in range(N_CORES)]


def assemble(X, results):
    """Stitch device outputs (atom-major bf16 [3, rows, 3]) into the
    full [N, 14, 3] f32 array."""
    out = np.array(X, dtype=np.float32, copy=True)
    Ys = [np.asarray(results[c]["Y"]).astype(np.float32)
          for c in range(N_CORES)]
    Yall = np.concatenate(Ys, axis=1)  # [3, N_TOTAL, 3]
    out[:, 0, :] = Yall[0]
    out[:, 2, :] = Yall[1]
    out[:, 3, :] = Yall[2]
    return out


def kernel(X, batch_ids=None, max_len=None, **_unused):
    from concourse.bass_utils import run_bass_kernel_spmd

    X = np.asarray(X, dtype=np.float32)
    assert X.shape == (N_TOTAL, 14, 3), X.shape
    nc = _get_nc()
    res = run_bass_kernel_spmd(nc, make_in_maps(X), list(range(N_CORES))).results
    return assemble(X, res)


# revision 6
# speedup vs baseline: 1.7637x; 1.0483x over previous
"""Trainium2 Bass kernel for nn_BackboneModel (backbone frame rebuild).

The reference scatters rows into a padded [B, L, 14, 3] block, builds
Gram-Schmidt rigid frames from (N, CA, C), places ideal N/CA/C/O atoms,
and gathers the valid rows back.  Scatter followed by gather at the same
(batch_id, pos) indices is an identity permutation over the valid rows,
so the whole model is a pure per-row function of X[i]:

    e1 = normalize(C - CA)                      (normalize: v * rsqrt(|v|^2 + eps^2))
    e2 = normalize((N - CA) - ((N - CA).e1) e1)
    out[0] = -0.525*e1 + 1.363*e2 + CA          (N)
    out[1] = CA                                 (CA, passthrough)
    out[2] =  1.526*e1            + CA          (C)
    out[3] =  2.153*e1 - 1.062*e2 + CA          (O)
    out[4:14] = X[4:14]                         (passthrough)

Only atoms 0..2 (N, CA, C) feed any arithmetic, and only atoms 0, 2, 3
get new values.  The device reads a packed [rows, 9] f32 block (N, CA,
C) and writes a packed atom-major [3, rows, 3] bf16 block (outN, outC,
outO); CA and atoms 4..13 are stitched in on the host during unshard
(pure data movement).  Correctness gate is rel-L2 < 2e-2; bf16 rounding
of final coordinates contributes ~1e-3.

Numerics: the Gram-Schmidt rejection w = v - (v.e1)e1 amplifies error in
the projection scalar g by ~|v|/|w|, so g = (v.d1)/|d1|^2 uses the DVE
reciprocal-approx (18 bits) and w is formed in f32.  The unit scalings
rs1/rs2 only scale outputs, so the ACT table Sqrt is fine there.  The
reference's +eps^2 regularizer is dropped (inputs are randn; |d1|^2 is
never near eps^2 = 1e-6, relative effect ~1e-8).

Sharding: data-parallel, 8 equal contiguous row chunks of 98304 rows.
"""

import numpy as np

N_CORES = 8
N_TOTAL = 786432
N_CORE = N_TOTAL // N_CORES      # 98304 rows per core
P = 128                          # SBUF partitions
ROWS_PER_PART = N_CORE // P      # 768 rows per partition per core
TILE_SIZES = [256, 256, 256]
CIN = 9                          # N, CA, C xyz (f32)
EPS2 = 1e-6                      # unused on device; kept for reference

_NC = None


def _build_nc():
    import concourse.bacc as bacc
    import concourse.tile as tile
    from concourse import mybir

    f32 = mybir.dt.float32
    bf16 = mybir.dt.bfloat16
    MUL = mybir.AluOpType.mult
    ADD = mybir.AluOpType.add
    AX = mybir.AxisListType.X
    SQRT = mybir.ActivationFunctionType.Sqrt
    SQUARE = mybir.ActivationFunctionType.Square
    COPY = mybir.ActivationFunctionType.Copy

    nc = bacc.Bacc()
    X = nc.declare_dram_parameter("X", [N_CORE, CIN], f32, isOutput=False)
    Y = nc.declare_dram_parameter("Y", [3, N_CORE, 3], bf16, isOutput=True)

    def bcast(s, r):  # [P, r] per-row scalar -> [P, r, 3]
        return s[:, :, None].broadcast_to([P, r, 3])

    with tile.TileContext(nc) as tc:
        with tc.tile_pool(name="io", bufs=3) as io, \
             tc.tile_pool(name="v3", bufs=3) as v3, \
             tc.tile_pool(name="sc", bufs=3) as sc:

            def head(row_off, R):
                st = {"R": R, "off": row_off}
                T = st["T"] = io.tile([P, R, CIN], f32, tag="T", name="T")
                nc.sync.dma_start(
                    out=T,
                    in_=X[row_off:row_off + P * R, :].rearrange(
                        "(p r) c -> p r c", p=P))
                Na = T[:, :, 0:3]
                CAa = T[:, :, 3:6]
                Ca = T[:, :, 6:9]

                D1 = v3.tile([P, R, 3], f32, tag="d1", name="d1")
                V = v3.tile([P, R, 3], f32, tag="v", name="v")
                QQ = v3.tile([P, R, 6], f32, tag="qq", name="qq")
                T1 = v3.tile([P, R, 3], f32, tag="t1", name="t1")
                W = v3.tile([P, R, 3], f32, tag="w", name="w")
                SQ2 = v3.tile([P, R, 3], f32, tag="sq2", name="sq2")
                CAb = st["CAb"] = v3.tile([P, R, 3], bf16, tag="cab",
                                          name="cab")
                D1b = st["D1b"] = v3.tile([P, R, 3], bf16, tag="d1b",
                                          name="d1b")
                Wb = st["Wb"] = v3.tile([P, R, 3], bf16, tag="wb", name="wb")
                RS1d = st["RS1d"] = v3.tile([P, R, 3], bf16, tag="rs1d",
                                            name="rs1d")
                RS2d = st["RS2d"] = v3.tile([P, R, 3], bf16, tag="rs2d",
                                            name="rs2d")
                SD = sc.tile([P, R, 2], f32, tag="sd", name="sd")
                IS1 = sc.tile([P, R], f32, tag="is1", name="is1")
                RS1 = sc.tile([P, R], f32, tag="rs1", name="rs1")
                G = sc.tile([P, R], f32, tag="g", name="g")
                S2 = sc.tile([P, R], f32, tag="s2", name="s2")
                IS2 = sc.tile([P, R], f32, tag="is2", name="is2")
                RS2 = sc.tile([P, R], f32, tag="rs2", name="rs2")

                # d1 = C - CA, v = N - CA  (Pool engine)
                nc.gpsimd.tensor_sub(D1, Ca, CAa)
                nc.gpsimd.tensor_sub(V, Na, CAa)
                # bf16 shadows for the 2x tail (ACT)
                nc.scalar.activation(out=CAb, in_=CAa, func=COPY)
                nc.scalar.activation(out=D1b, in_=D1, func=COPY)

                # fused |d1|^2 and v.d1: QQ = [d1^2 | v*d1], one reduce
                nc.scalar.activation(out=QQ[:, :, 0:3], in_=D1, func=SQUARE)
                nc.vector.tensor_mul(QQ[:, :, 3:6], V, D1)
                nc.vector.reduce_sum(
                    out=SD, in_=QQ.rearrange("p r (a c) -> p r a c", a=2),
                    axis=AX)
                nc.vector.reciprocal_approx_fast(out=IS1, in_=SD[:, :, 0])
                nc.scalar.activation(out=RS1, in_=IS1, func=SQRT)
                # rs1 materialized dense bf16 (stride-0 reads are free on ACT)
                nc.scalar.activation(out=RS1d, in_=bcast(RS1, R), func=COPY)

                # w = v - (dot * is1) d1   (f32 rejection; W sub on Pool)
                nc.vector.tensor_mul(G, SD[:, :, 1], IS1)
                nc.vector.tensor_mul(T1, D1, bcast(G, R))
                nc.gpsimd.tensor_sub(W, V, T1)

                # rs2 = rsqrt(|w|^2), materialized dense bf16
                nc.scalar.activation(out=SQ2, in_=W, func=SQUARE)
                nc.vector.reduce_sum(out=S2, in_=SQ2, axis=AX)
                nc.vector.reciprocal_approx_fast(out=IS2, in_=S2)
                nc.scalar.activation(out=RS2, in_=IS2, func=SQRT)
                nc.scalar.activation(out=RS2d, in_=bcast(RS2, R), func=COPY)
                nc.scalar.activation(out=Wb, in_=W, func=COPY)
                return st

            def tail(st):
                R = st["R"]
                E1 = v3.tile([P, R, 3], bf16, tag="e1", name="e1")
                E2 = v3.tile([P, R, 3], bf16, tag="e2", name="e2")
                SA = v3.tile([P, R, 3], bf16, tag="sa", name="sa")
                SB = v3.tile([P, R, 3], bf16, tag="sb", name="sb")
                TN = v3.tile([P, R, 3], bf16, tag="tn", name="tn")
                TO = v3.tile([P, R, 3], bf16, tag="to", name="to")
                OUT = io.tile([P, 3, R, 3], bf16, tag="OUT", name="OUT")
                CAb = st["CAb"]

                # e1/e2 as dense bf16 x bf16 products (2x mode)
                nc.vector.tensor_mul(E1, st["D1b"], st["RS1d"])
                nc.vector.tensor_mul(E2, st["Wb"], st["RS2d"])
                # atom-major OUT: [:,0]=N, [:,1]=C, [:,2]=O.
                # TS prescale runs 4x, bf16 TT add runs 2x.
                nc.vector.tensor_scalar_mul(out=SA, in0=E1, scalar1=1.526)
                nc.vector.tensor_add(OUT[:, 1], SA, CAb)
                nc.vector.tensor_scalar_mul(out=SB, in0=E2, scalar1=1.363)
                nc.vector.tensor_add(TN, SB, CAb)
                nc.vector.tensor_scalar_mul(out=SA, in0=E1, scalar1=-0.525)
                nc.vector.tensor_add(OUT[:, 0], SA, TN)
                nc.vector.tensor_scalar_mul(out=SB, in0=E2, scalar1=-1.062)
                nc.vector.tensor_add(TO, SB, CAb)
                nc.vector.tensor_scalar_mul(out=SA, in0=E1, scalar1=2.153)
                nc.vector.tensor_add(OUT[:, 2], SA, TO)
                nc.scalar.dma_start(
                    out=Y[:, st["off"]:st["off"] + P * st["R"], :].rearrange(
                        "a (p r) c -> p a r c", p=P),
                    in_=OUT)

            offs = []
            o = 0
            for R in TILE_SIZES:
                offs.append(o)
                o += P * R
            assert o == N_CORE
            sts = [head(offs[i], R) for i, R in enumerate(TILE_SIZES)]
            for st in sts:
                tail(st)
    nc.finalize()
    return nc


def _get_nc():
    global _NC
    if _NC is None:
        _NC = _build_nc()
    return _NC


def make_in_maps(X):
    """Pack (N, CA, C) as contiguous [N_CORE, 9] f32 shards per core."""
    X = np.asarray(X, dtype=np.float32)
    A = np.ascontiguousarray(X[:, 0:3, :]).reshape(N_TOTAL, CIN)
    shards = A.reshape(N_CORES, N_CORE, CIN)
    return [{"X": shards[c]} for c in range(N_CORES)]


def assemble(X, results):
    """Stitch device outputs (atom-major bf16 [3, rows, 3]) into the
    full [N, 14, 3] f32 array."""
    out = np.array(X, dtype=np.float32, copy=True)
    Ys = [np.asarray(results[c]["Y"]).astype(np.float32)
          for c in range(N_CORES)]
    Yall = np.concatenate(Ys, axis=1)  # [3, N_TOTAL, 3]
    out[:, 0, :] = Yall[0]
    out[:, 2, :] = Yall[1]
    out[:, 3, :] = Yall[2]
    return out


def kernel(X, batch_ids=None, max_len=None, **_unused):
    from concourse.bass_utils import run_bass_kernel_spmd

    X = np.asarray(X, dtype=np.float32)
    assert X.shape == (N_TOTAL, 14, 3), X.shape
    nc = _get_nc()
    res = run_bass_kernel_spmd(nc, make_in_maps(X), list(range(N_CORES))).results
    return assemble(X, res)
